# revision 20
# baseline (speedup 1.0000x reference)
"""GIN message-passing encoder (3 layers) on 8 Trainium2 NeuronCores.

Problem: x_{l+1} = relu(BN(relu((x + agg(x)) @ W1 + b1) @ W2 + b2)),
agg[b, d] = sum over edges (s -> d) of x[b, s]; output = stack of the 3
layer outputs, shape [3, 16, 1024, 256].

Strategy
--------
- Data parallel over batch: B=16 split as 2 batch elements per core.
- The scatter-add is a dense matmul against a host-built (N x N) matrix
  Bm[s, d] = I[s, d] + multiplicity(edge s -> d); the +x of GIN(eps=0)
  is the identity fold.
- Eval-mode BatchNorm is folded into W2/b2 on the host.
- step1 runs in fp8e4 DoubleRow mode (K=256 per instruction, halving
  the number of PSUM accumulation passes vs f32r's K=128): Bm is exact
  in fp8 (small ints); x is quantized e4m3 single-pass (measured rel
  err ~1.1e-2 vs the 2e-2 gate; hi/lo compensation would double the
  passes and erase the speedup). x0 is quantized host-side; x1/x2
  on-device (second ACT relu from step3's PSUM with fp8 output).
- b2 bias enters step3's PSUM via a ones-matmul (lhsT=ones[128,128],
  rhs has b2' on partition 0, zeros elsewhere), so ACT applies relu
  straight from PSUM; no DVE broadcast-add.
- MLP matmuls stay float32r (full PE rate at moving-free >= 256).
- Input DMAs are host-preswizzled (straight per-partition runs) and
  issued across sync/vector/scalar/gpsimd queues to cut the ~620ns
  per-DMA issue serialization; outputs stream on the sync queue.
"""

import os

import numpy as np

BN_EPS = 1e-5

B, N, F = 16, 1024, 256
L = 3
NCORES = 8
BPC = B // NCORES  # batch elements per core
P = 128
NT = N // P   # 8 node tiles
FT = F // P   # 2 feature tiles
KK = N // 256  # 4 double-chunks of the contraction dim (DoubleRow K=256)
HALF = 512    # moving free-dim chunk
NH = N // HALF  # 2 halves of the node dim

_cache: dict = {}


def _build_nc():
    import concourse.bacc as bacc
    import concourse.mybir as mybir
    import concourse.tile as tile

    F32 = mybir.dt.float32
    F32R = mybir.dt.float32r
    F8 = mybir.dt.float8e4
    Relu = mybir.ActivationFunctionType.Relu
    Copy = mybir.ActivationFunctionType.Copy
    Alu = mybir.AluOpType
    DR = mybir.MatmulPerfMode.DoubleRow

    nc = bacc.Bacc()

    x0hi_d = nc.dram_tensor("x0hi", [BPC, P, KK, 2, F], F8, kind="ExternalInput")
    bm_d = nc.dram_tensor("bm", [P, KK, 2, N], F8, kind="ExternalInput")
    w1_d = nc.dram_tensor("w1", [P, L, FT, F], F32R, kind="ExternalInput")
    w2_d = nc.dram_tensor("w2", [P, L, FT, F], F32R, kind="ExternalInput")
    b1_d = nc.dram_tensor("b1", [P, L * FT], F32, kind="ExternalInput")
    b2_d = nc.dram_tensor("b2", [P, L, 2 * F], F32R, kind="ExternalInput")
    ones_d = nc.dram_tensor("ones", [P, P], F32R, kind="ExternalInput")
    out_d = nc.dram_tensor("out", [L, BPC, N, F], F32R, kind="ExternalOutput")

    with tile.TileContext(nc) as tc:
        with (
            tc.tile_pool(name="const", bufs=1) as cpool,
            tc.tile_pool(name="x8", bufs=2) as xpool,
            tc.tile_pool(name="m0", bufs=2) as wpool,
            tc.tile_pool(name="h1", bufs=2) as hpool,
            tc.tile_pool(name="yt", bufs=6) as ypool,
            tc.tile_pool(name="pm0", bufs=2, space="PSUM") as pm0,
            tc.tile_pool(name="ph1", bufs=2, space="PSUM") as ph1,
            tc.tile_pool(name="py", bufs=2, space="PSUM") as py,
        ):
            bm_sb = cpool.tile([P, KK, 2, N], F8)
            w1_sb = cpool.tile([P, L, FT, F], F32R)
            w2_sb = cpool.tile([P, L, FT, F], F32R)
            b1_sb = cpool.tile([P, L * FT], F32)
            b2z_sb = cpool.tile([P, L, 2 * F], F32R)
            ones_sb = cpool.tile([P, P], F32R)

            xhi = xpool.tile([P, BPC, KK, 2, F], F8, tag="xhi")

            # Input DMAs spread across the sync + scalar queues (issue is
            # ~700ns each; gpsimd SWDGE would add a ~10us drain). Host
            # pre-swizzles so every transfer is straight contiguous
            # per-partition runs. First matmul needs bm[kk0] + xhi[0][kk0]
            # only, so those go first in small pieces.
            nc.sync.dma_start(bm_sb[:, 0:1], bm_d[:, 0:1])
            nc.sync.dma_start(bm_sb[:, 1:2], bm_d[:, 1:2])
            nc.sync.dma_start(bm_sb[:, 2:4], bm_d[:, 2:4])
            nc.scalar.dma_start(xhi[:, 0, 0:1], x0hi_d[0, :, 0:1])
            nc.scalar.dma_start(xhi[:, 0, 1:4], x0hi_d[0, :, 1:4])
            nc.scalar.dma_start(w1_sb[:], w1_d[:])
            nc.scalar.dma_start(xhi[:, 1], x0hi_d[1])
            nc.scalar.dma_start(b1_sb[:], b1_d[:])
            nc.scalar.dma_start(ones_sb[:], ones_d[:])
            nc.scalar.dma_start(b2z_sb[:], b2_d[:])
            nc.scalar.dma_start(w2_sb[:], w2_d[:])

            for l in range(L):
                last = l == L - 1
                if not last:
                    nxhi = xpool.tile([P, BPC, KK, 2, F], F8, tag="xhi")
                for b in range(BPC):
                    # ---- step 1: m0T = (A + I) @ x_q, fp8 DoubleRow ----
                    m0t = wpool.tile([P, FT, N], F32R, tag="m0t")
                    for ft in range(FT):
                        for half in range(NH):
                            ps = pm0.tile([P, HALF], F32, tag="pm0")
                            for kk in range(KK):
                                nc.tensor.matmul(
                                    ps[:],
                                    xhi[:, b, kk, :, ft * P:(ft + 1) * P],
                                    bm_sb[:, kk, :,
                                          half * HALF:(half + 1) * HALF],
                                    start=(kk == 0),
                                    stop=(kk == KK - 1),
                                    perf_mode=DR,
                                )
                            nc.vector.tensor_copy(
                                m0t[:, ft, half * HALF:(half + 1) * HALF],
                                ps[:],
                            )
                    # ---- step 2: h1T = relu(W1^T-contract @ m0T + b1) ----
                    h1t = hpool.tile([P, FT, N], F32R, tag="h1t")
                    for gt in range(FT):
                        ps2 = ph1.tile([P, NH * HALF], F32, tag="ph1")
                        for half in range(NH):
                            for fk in range(FT):
                                nc.tensor.matmul(
                                    ps2[:, half * HALF:(half + 1) * HALF],
                                    w1_sb[:, l, fk, gt * P:(gt + 1) * P],
                                    m0t[:, fk, half * HALF:(half + 1) * HALF],
                                    start=(fk == 0),
                                    stop=(fk == FT - 1),
                                )
                        nc.scalar.activation(
                            h1t[:, gt, :],
                            ps2[:],
                            Relu,
                            bias=b1_sb[:, l * FT + gt:l * FT + gt + 1],
                        )
                    # ---- step 3: y = relu(h1 @ W2' + b2') -> out + next x ----
                    for tp in range(NT // 2):
                        ps3 = py.tile([P, 2, F], F32, tag="py")
                        # one 512-wide ones-matmul seeds b2' into both
                        # j-halves of the PSUM tile
                        nc.tensor.matmul(
                            ps3[:, :, :], ones_sb[:], b2z_sb[:, l, :],
                            start=True, stop=False, skip_group_check=True,
                        )
                        for j in range(2):
                            nt = 2 * tp + j
                            for gk in range(FT):
                                nc.tensor.matmul(
                                    ps3[:, j, :],
                                    h1t[:, gk, nt * P:(nt + 1) * P],
                                    w2_sb[:, l, gk, :],
                                    start=False,
                                    stop=(gk == FT - 1),
                                    skip_group_check=True,
                                )
                        ynorm = ypool.tile([P, 2, F], F32R, tag="y")
                        nc.scalar.activation(ynorm[:], ps3[:], Relu)
                        if last and b == BPC - 1 and tp == NT // 2 - 1:
                            # tail: split the final store in two
                            for j in range(2):
                                nt = 2 * tp + j
                                nc.sync.dma_start(
                                    out_d[l, b, nt * P:(nt + 1) * P, :]
                                    .rearrange("(t p) f -> p t f", p=P),
                                    ynorm[:, j:j + 1, :],
                                )
                        else:
                            nc.sync.dma_start(
                                out_d[l, b,
                                      2 * tp * P:(2 * tp + 2) * P, :].rearrange(
                                    "(t p) f -> p t f", p=P
                                ),
                                ynorm[:],
                            )
                        if not last:
                            # fp8 quantize from the already-relu'd SBUF copy
                            nc.scalar.activation(
                                nxhi[:, b, tp, :, :], ynorm[:], Copy
                            )
                if not last:
                    xhi = nxhi

    nc.finalize()
    return nc


def kernel(h, edge_index, W1, b1, W2, b2, gamma, beta, run_mean, run_var):
    import ml_dtypes
    from concourse.bass_utils import run_bass_kernel_spmd

    f8 = ml_dtypes.float8_e4m3

    h = np.asarray(h, dtype=np.float32)
    edge_index = np.asarray(edge_index)
    W1 = np.asarray(W1, dtype=np.float32)
    b1 = np.asarray(b1, dtype=np.float32)
    W2 = np.asarray(W2, dtype=np.float32)
    b2 = np.asarray(b2, dtype=np.float32)
    gamma = np.asarray(gamma, dtype=np.float32)
    beta = np.asarray(beta, dtype=np.float32)
    run_mean = np.asarray(run_mean, dtype=np.float32)
    run_var = np.asarray(run_var, dtype=np.float32)

    # host-side preprocessing
    src = edge_index[0].astype(np.int64)
    dst = edge_index[1].astype(np.int64)
    bm = np.zeros((N, N), dtype=np.float32)
    np.add.at(bm, (src, dst), 1.0)
    bm[np.arange(N), np.arange(N)] += 1.0
    # fp8 exact for small integer counts; DoubleRow layout [P, KK, 2, N]
    bm8 = np.ascontiguousarray(
        bm.astype(f8).reshape(KK, 2, P, N).transpose(2, 0, 1, 3)
    )

    # x0 quantized to fp8 on the host, swizzled to [B, P, KK, 2, F]
    xhi8s = np.ascontiguousarray(
        h.astype(f8).reshape(B, KK, 2, P, F).transpose(0, 3, 1, 2, 4)
    )

    inv = (gamma / np.sqrt(run_var + BN_EPS)).astype(np.float32)      # [L, F]
    w2f = (W2 * inv[:, None, :]).astype(np.float32)                   # [L, F, F]
    b2f = (b2 * inv + beta - run_mean * inv).astype(np.float32)       # [L, F]

    # weights swizzled to [P, L, FT, F] (contraction chunk on partitions)
    w1s = np.ascontiguousarray(W1.reshape(L, FT, P, F).transpose(2, 0, 1, 3))
    w2s = np.ascontiguousarray(w2f.reshape(L, FT, P, F).transpose(2, 0, 1, 3))
    # b1 as per-partition scalars: [P, L*FT]
    b1r = np.ascontiguousarray(
        b1.reshape(L, FT, P).transpose(2, 0, 1).reshape(P, L * FT)
    )
    # b2' (duplicated pair) on partition 0 only; the 512-wide ones-matmul
    # broadcasts it into both halves of step3's PSUM tile
    b2r = np.zeros((P, L, 2 * F), dtype=np.float32)
    b2r[0] = np.concatenate([b2f, b2f], axis=1)
    ones_h = np.ones((P, P), dtype=np.float32)

    if "nc" not in _cache:
        _cache["nc"] = _build_nc()
    nc = _cache["nc"]

    in_maps = []
    for c in range(NCORES):
        in_maps.append({
            "x0hi": np.ascontiguousarray(xhi8s[c * BPC:(c + 1) * BPC]),
            "bm": bm8,
            "w1": w1s,
            "w2": w2s,
            "b1": b1r,
            "b2": b2r,
            "ones": ones_h,
        })

    trace = os.environ.get("KERNEL_TRACE") == "1"
    res = run_bass_kernel_spmd(
        nc, in_maps, core_ids=list(range(NCORES)), trace=trace
    )
    _cache["last_results"] = res
    return np.concatenate([r["out"] for r in res.results], axis=1)
